# revision 6
# baseline (speedup 1.0000x reference)
"""CoAttention Trainium2 Bass kernel.

Sharding: data-parallel over batch B=8 across the 8 NeuronCores (one batch
element per core); the CxC projection weights are replicated.

Per-core math (x1, x2 are [C, L] channel-major slices of one batch element):
  qT = w_q @ x1 + b_q          [C, L]   (c_out on partitions)
  kT = w_k @ x2 + b_k          [C, L]
  v1 = x1^T @ w_v1^T + b_v1    [L, C]   (l on partitions)
  v2 = x2^T @ w_v2^T + b_v2    [L, C]
  S[q,k] = (qT^T kT)[q,k]                      (attn logits * sqrt(C))
  pass A: E_T = exp(S^T/sqrt(C)) tiles [k,q];  vk = E_T^T @ v2 / colsum_k
          out1 = (LN(vk + x1^T))^T
  pass B: E_S = exp(S/sqrt(C))  tiles [q,k];   vq = E_S^T @ v1 / colsum_q
          out2 = (LN(vq + x2^T))^T
Softmax max-subtraction is skipped: logits ~ N(0,1) (|logit| < ~6), so
exp() is numerically safe in fp32 and the result is mathematically
identical to jax.nn.softmax.

Matmuls run as float32r (full-rate fp32 mode, ~1 cycle/row for N>=256);
PE-mode transposes handle all [l,c]<->[c,l] layout changes (fp32 has no
DMA-transpose). Softmax denominators are computed with ones-vector
matmuls on the PE (partition-dim sums), all row-wise stats on DVE.
"""

import sys

import numpy as np

try:
    import concourse.bass as bass  # noqa: F401
except ImportError:  # grading env may not have it on sys.path
    sys.path.insert(0, "/opt/trn_rl_repo")

import concourse.bass as bass
import concourse.tile as tile
from concourse import bacc, mybir
from concourse.bass_utils import run_bass_kernel_spmd

C = 512
L_FULL = 2048
B = 8
NCORES = 8
P = 128
EPS = 1e-5
INV_SQRT_C = 1.0 / float(np.sqrt(C))
F32 = mybir.dt.float32
F32R = mybir.dt.float32r
CT = C // P  # 4 c-chunks
QCH = 256  # q-chunk (free-dim block) per pass iteration

Alu = mybir.AluOpType
Act = mybir.ActivationFunctionType


def _r(ap):
    """bitcast fp32 -> fp32r (same bytes) for DMA into fp32r tiles."""
    return ap.bitcast(F32R)


def _co_pass(nc, pools, L, lhs_sb, rhs_sb, v_sb, xres_view, out_view, consts):
    """One attention stream.

    eblk[p=r_idx, f=m_idx] = exp(lhs_row(r) . rhs_row(m) / sqrt(C)) where
    lhs/rhs are [c, l] projections. Output rows are the m (free) index:
      out[m, c] = LN( (sum_r eblk[r,m] * v[r,c]) / (sum_r eblk[r,m]) + xres[m, c] )
    stored transposed into out_view ([p c-slice, g, l=m]).
    """
    big, wsmall, work, vec, ps_mm, ps_small, ps_tr, ps_pv, ps_res = pools
    ident, ones_col, ones_11, g_sb, bb_sb, eps_sb = consts
    LT = L // P  # partition tiles along l
    for ci in range(L // QCH):
        q0 = ci * QCH
        xres = wsmall.tile([P, CT, QCH], F32, tag="B")
        nc.sync.dma_start(out=xres[:], in_=xres_view[:, :, q0 : q0 + QCH])
        eblk = big.tile([P, LT, QCH], F32R, tag="A")
        for kt in range(LT):
            ps = ps_mm.tile([P, QCH], F32, tag="ps_mm")
            for c in range(CT):
                nc.tensor.matmul(
                    ps[:],
                    lhsT=lhs_sb[:, c, kt * P : (kt + 1) * P],
                    rhs=rhs_sb[:, c, q0 : q0 + QCH],
                    start=(c == 0),
                    stop=(c == CT - 1),
                )
            nc.scalar.activation(
                out=eblk[:, kt, :], in_=ps[:], func=Act.Exp, scale=INV_SQRT_C
            )
        # softmax denominators for this q-chunk: column sums over all l rows
        ps_s = ps_small.tile([P, QCH], F32, tag="ps_srow")
        for kt in range(LT):
            nc.tensor.matmul(
                ps_s[0:1, :],
                lhsT=ones_col[:],
                rhs=eblk[:, kt, :],
                start=(kt == 0),
                stop=(kt == LT - 1),
            )
        srow = vec.tile([1, QCH], F32, tag="srow")
        nc.vector.tensor_copy(out=srow[:], in_=ps_s[0:1, :])
        for qs in range(QCH // P):
            qsl = slice(qs * P, (qs + 1) * P)
            # [1, 128] denominator slice -> per-partition [128, 1]
            ps_t = ps_tr.tile([P, P], F32, tag="ps_tr")
            nc.tensor.matmul(
                ps_t[:, 0:1], lhsT=srow[0:1, qsl], rhs=ones_11[:], start=True, stop=True
            )
            rec = vec.tile([P, 1], F32, tag="rec")
            nc.vector.reciprocal(out=rec[:], in_=ps_t[:, 0:1])
            # attention-weighted values: accumulate over all l rows
            ps_v = ps_pv.tile([P, C], F32, tag="ps_pv")
            for kt in range(LT):
                nc.tensor.matmul(
                    ps_v[:],
                    lhsT=eblk[:, kt, qsl],
                    rhs=v_sb[:, kt, :],
                    start=(kt == 0),
                    stop=(kt == LT - 1),
                )
            # residual x^T block via PE transpose
            ps_x = ps_res.tile([P, C], F32, tag="ps_res")
            for g in range(CT):
                nc.tensor.transpose(
                    ps_x[:, g * P : (g + 1) * P], xres[:, g, qsl], ident[:]
                )
            u = work.tile([P, C], F32, tag="u")
            nc.vector.tensor_scalar_mul(out=u[:], in0=ps_v[:], scalar1=rec[:])
            nc.vector.tensor_add(out=u[:], in0=u[:], in1=ps_x[:])
            # layernorm over free dim (c)
            stats = vec.tile([P, 6], F32, tag="stats")
            nc.vector.bn_stats(out=stats[:], in_=u[:])
            mv = vec.tile([P, 2], F32, tag="mv")
            nc.vector.bn_aggr(out=mv[:], in_=stats[:])
            rstd = vec.tile([P, 1], F32, tag="rstd")
            nc.scalar.activation(
                out=rstd[:], in_=mv[:, 1:2], func=Act.Sqrt, bias=eps_sb[:], scale=1.0
            )
            nc.vector.reciprocal(out=rstd[:], in_=rstd[:])
            nc.vector.tensor_scalar(
                out=u[:],
                in0=u[:],
                scalar1=mv[:, 0:1],
                scalar2=rstd[:],
                op0=Alu.subtract,
                op1=Alu.mult,
            )
            # transpose back to [c, l]; gamma/beta are per-partition there
            ost = work.tile([P, CT, P], F32, tag="ostage")
            for g in range(CT):
                ps_o = ps_tr.tile([P, P], F32, tag="ps_tr")
                nc.tensor.transpose(ps_o[:], u[:, g * P : (g + 1) * P], ident[:])
                nc.vector.tensor_scalar(
                    out=ost[:, g, :],
                    in0=ps_o[:],
                    scalar1=g_sb[:, g : g + 1],
                    scalar2=bb_sb[:, g : g + 1],
                    op0=Alu.mult,
                    op1=Alu.add,
                )
            nc.sync.dma_start(
                out=out_view[:, :, q0 + qs * P : q0 + (qs + 1) * P], in_=ost[:]
            )


def _build(L=L_FULL):
    nc = bacc.Bacc(
        "TRN2",
        target_bir_lowering=False,
        debug=False,
        enable_asserts=False,
        num_devices=NCORES,
    )
    dram = lambda n, s, kind: nc.dram_tensor(n, s, F32, kind=kind).ap()
    x1d = dram("x1", [C, L], "ExternalInput")
    x2d = dram("x2", [C, L], "ExternalInput")
    wd = {n: dram(n, [C, C], "ExternalInput") for n in ("w_q", "w_k", "w_v1", "w_v2")}
    bd = {n: dram(n, [C], "ExternalInput") for n in ("b_q", "b_k", "b_v1", "b_v2")}
    gd = dram("ln_gamma", [C], "ExternalInput")
    betad = dram("ln_beta", [C], "ExternalInput")
    identd = dram("ident", [P, P], "ExternalInput")
    onesd = dram("ones_const", [P], "ExternalInput")
    out1d = dram("out1", [C, L], "ExternalOutput")
    out2d = dram("out2", [C, L], "ExternalOutput")

    x1v = x1d.rearrange("(t p) l -> p t l", p=P)
    x2v = x2d.rearrange("(t p) l -> p t l", p=P)
    o1v = out1d.rearrange("(g p) l -> p g l", p=P)
    o2v = out2d.rearrange("(g p) l -> p g l", p=P)

    LT = L // P
    with tile.TileContext(nc) as tc:
        with (
            tc.tile_pool(name="res", bufs=1) as res,  # long-lived projections
            tc.tile_pool(name="big", bufs=2) as big,  # xproj / eblk
            tc.tile_pool(name="wsmall", bufs=2) as wsmall,  # wT / xres
            tc.tile_pool(name="wn", bufs=2) as wnp,
            tc.tile_pool(name="work", bufs=3) as work,
            tc.tile_pool(name="vec", bufs=3) as vec,
            tc.tile_pool(name="singles", bufs=1) as singles,
            tc.tile_pool(name="ps_mm", bufs=2, space="PSUM") as ps_mm,
            tc.tile_pool(name="ps_small", bufs=1, space="PSUM") as ps_small,
            tc.tile_pool(name="ps_tr", bufs=2, space="PSUM") as ps_tr,
            tc.tile_pool(name="ps_pv", bufs=2, space="PSUM") as ps_pv,
            tc.tile_pool(name="ps_res", bufs=1, space="PSUM") as ps_res,
        ):
            # constants
            ident = singles.tile([P, P], F32)
            nc.sync.dma_start(out=ident[:], in_=identd)
            ones_col = singles.tile([P, 1], F32R)
            nc.sync.dma_start(out=ones_col[:], in_=_r(onesd.unsqueeze(1)))
            ones_row = singles.tile([1, P], F32R)
            nc.sync.dma_start(out=ones_row[:], in_=_r(onesd.unsqueeze(0)))
            ones_11 = singles.tile([1, 1], F32)
            nc.vector.memset(ones_11[:], 1.0)
            eps_sb = singles.tile([P, 1], F32)
            nc.vector.memset(eps_sb[:], EPS)
            g_sb = singles.tile([P, CT], F32)
            nc.sync.dma_start(out=g_sb[:], in_=gd.rearrange("(t p) -> p t", p=P))
            bb_sb = singles.tile([P, CT], F32)
            nc.sync.dma_start(out=bb_sb[:], in_=betad.rearrange("(t p) -> p t", p=P))
            bq_sb = singles.tile([P, CT], F32)
            nc.sync.dma_start(out=bq_sb[:], in_=bd["b_q"].rearrange("(t p) -> p t", p=P))
            bk_sb = singles.tile([P, CT], F32)
            nc.sync.dma_start(out=bk_sb[:], in_=bd["b_k"].rearrange("(t p) -> p t", p=P))
            bv1_row = singles.tile([1, C], F32R)
            nc.sync.dma_start(out=bv1_row[:], in_=_r(bd["b_v1"].unsqueeze(0)))
            bv2_row = singles.tile([1, C], F32R)
            nc.sync.dma_start(out=bv2_row[:], in_=_r(bd["b_v2"].unsqueeze(0)))

            # long-lived projection outputs
            qT = res.tile([P, CT, L], F32R, tag="qT")
            kT = res.tile([P, CT, L], F32R, tag="kT")
            v1 = res.tile([P, LT, C], F32R, tag="v1")
            v2 = res.tile([P, LT, C], F32R, tag="v2")

            def transpose_w(wname):
                """native w [d, c] -> wT [c-tile, 4, d] in SBUF."""
                wT = wsmall.tile([P, CT, C], F32R, tag="B")
                wv = wd[wname].rearrange("(t p) c -> p t c", p=P)
                for t in range(CT):  # d-tile
                    wn_t = wnp.tile([P, C], F32, tag="wn")
                    nc.sync.dma_start(out=wn_t[:], in_=wv[:, t, :])
                    for s in range(CT):  # c-slice
                        ps = ps_tr.tile([P, P], F32, tag="ps_tr")
                        nc.tensor.transpose(ps[:], wn_t[:, s * P : (s + 1) * P], ident[:])
                        nc.vector.tensor_copy(
                            out=wT[:, s, t * P : (t + 1) * P], in_=ps[:]
                        )
                return wT

            def project(xview, wTqk, b_qk_sb, tT, wTv, bv_row, vout):
                """From x [c,l] produce tT=[w@x+b] ([c_out,l]) and
                v=[x^T w^T + b] ([l, c_out])."""
                for n in range(L // C):  # 512-wide l chunk
                    xp = big.tile([P, CT, C], F32R, tag="A")
                    nc.sync.dma_start(
                        out=xp[:], in_=_r(xview[:, :, n * C : (n + 1) * C])
                    )
                    for m in range(CT):
                        ps = ps_mm.tile([P, C], F32, tag="ps_mm")
                        for c in range(CT):
                            nc.tensor.matmul(
                                ps[:],
                                lhsT=wTqk[:, c, m * P : (m + 1) * P],
                                rhs=xp[:, c, :],
                                start=(c == 0),
                                stop=(c == CT - 1),
                            )
                        nc.vector.tensor_scalar(
                            out=tT[:, m, n * C : (n + 1) * C],
                            in0=ps[:],
                            scalar1=b_qk_sb[:, m : m + 1],
                            scalar2=None,
                            op0=Alu.add,
                        )
                    for lt in range(CT):  # l-tile within chunk
                        ps = ps_mm.tile([P, C], F32, tag="ps_mm")
                        for c in range(CT):
                            nc.tensor.matmul(
                                ps[:],
                                lhsT=xp[:, c, lt * P : (lt + 1) * P],
                                rhs=wTv[:, c, :],
                                start=(c == 0),
                                stop=False,
                            )
                        nc.tensor.matmul(
                            ps[:],
                            lhsT=ones_row[:],
                            rhs=bv_row[:],
                            start=False,
                            stop=True,
                        )
                        nc.vector.tensor_copy(out=vout[:, n * CT + lt, :], in_=ps[:])

            wkT = transpose_w("w_k")
            wv2T = transpose_w("w_v2")
            project(x2v, wkT, bk_sb, kT, wv2T, bv2_row, v2)
            wqT = transpose_w("w_q")
            wv1T = transpose_w("w_v1")
            project(x1v, wqT, bq_sb, qT, wv1T, bv1_row, v1)

            pools = (big, wsmall, work, vec, ps_mm, ps_small, ps_tr, ps_pv, ps_res)
            consts = (ident, ones_col, ones_11, g_sb, bb_sb, eps_sb)
            # pass A: rows k (lhs), cols q (rhs) -> out1 = LN(vk + x1^T)^T
            _co_pass(nc, pools, L, kT, qT, v2, x1v, o1v, consts)
            # pass B: rows q, cols k -> out2 = LN(vq + x2^T)^T
            _co_pass(nc, pools, L, qT, kT, v1, x2v, o2v, consts)

    nc.compile()
    return nc


_NC_CACHE = {}


def _get_nc(L=L_FULL):
    if L not in _NC_CACHE:
        _NC_CACHE[L] = _build(L)
    return _NC_CACHE[L]


def _in_maps(inputs):
    arrs = {k: np.ascontiguousarray(np.asarray(v), dtype=np.float32) for k, v in inputs.items()}
    eye = np.eye(P, dtype=np.float32)
    maps = []
    for b in range(NCORES):
        m = {"x1": arrs["x1"][b], "x2": arrs["x2"][b], "ident": eye,
             "ones_const": np.ones(P, dtype=np.float32)}
        for n in ("w_q", "w_k", "w_v1", "w_v2", "b_q", "b_k", "b_v1", "b_v2"):
            m[n] = arrs[n]
        m["ln_gamma"] = arrs["ln_gamma"]
        m["ln_beta"] = arrs["ln_beta"]
        maps.append(m)
    return maps


def _run(inputs, trace=False):
    nc = _get_nc()
    res = run_bass_kernel_spmd(nc, _in_maps(inputs), list(range(NCORES)), trace=trace)
    out1 = np.stack([r_["out1"] for r_ in res.results])
    out2 = np.stack([r_["out2"] for r_ in res.results])
    return (out1, out2), res


def kernel(**inputs):
    (out1, out2), _ = _run(inputs)
    return out1, out2
